# revision 2
# baseline (speedup 1.0000x reference)
"""Trainium2 Bass kernel for nn_ExpandingLinear.

Reference computation (B=8192, F0=2048, E1=E2=256, O=1024, F2=2560):
    h1 = concat([x, relu(x[:, e1_parent] * e1_w)], 1)          # [B, 2304]
    h2 = concat([h1, relu(h1[:, e2_parent] * e2_w)], 1)        # [B, 2560]
    W  = scatter_add(zeros(O, F2), (w_rows, w_cols), w_vals)
    b  = scatter_add(zeros(O,), b_idx, b_vals)
    out = h2 @ W.T + b                                          # [B, O]

Algebraic reduction done on the host (weights only):
    relu(x * w) == w * relu(sign(w) * x) for scalar w, so every embed output
    column is (nonneg scalar) * relu(s * x[:, c]) for some source column c and
    sign s.  Folding each embed column's contribution through W gives

        out = x @ W0t + relu(S ⊙ xg) @ A + 1·bias

    where W0t = W[:, :2048].T, xg = the <=511 distinct (c, s) source columns,
    A is a small host-folded matrix, and the all-ones lhsT row adds the bias.

Device kernel (SPMD over 8 cores, batch-sharded 1024 rows/core):
    Dense [1024 x 2560] @ [2560 x 1024] bf16 matmul per core, scheduled as
    two passes of 8 PSUM-resident accumulation groups (full-K accumulation,
    no intermediate PSUM->SBUF adds):
      - input streams split across the sync (lhsT) and scalar (weights) DMA
        queues; first k-tile in small chunks so the PE starts early
      - memset-sourced warm-up matmuls flip the HAM clock gate to 2.4 GHz
        with no DMA dependency
      - pass 1 (m0-3 x n0-1): k-major over kt 0..15 (consumes tiles in DMA
        arrival order), then group-major endgame over kt 16..19 so the 8
        groups finish staggered and their drains/stores overlap the stream
      - pass 2 (m4-7): fully group-major (everything resident), outputs
        drain + store every ~4.3 us -> tiny tail
      - drains convert PSUM fp32 -> bf16 (half the DVE + store bytes);
        host upcasts to fp32
"""

import numpy as np

import concourse.bass as bass
import concourse.tile as tile
from concourse import bacc, mybir
from concourse.bass_utils import run_bass_kernel_spmd

B, F0, E1, E2, O = 8192, 2048, 256, 256, 1024
F1 = F0 + E1
F2 = F1 + E2
N_CORES = 8
BS = B // N_CORES          # 1024 batch rows per core
P = 128                    # partitions
KT_X = F0 // P             # 16 k-tiles of raw x
N_HALF = 512               # matmul moving free dim (fp32 PSUM bank limit)
MT = BS // P               # 8 m-tiles
WARMUP_MMS = 6

# matmul operand dtype (bfloat16: 1 cycle/row + half the DMA bytes, ~2e-3)
MATMUL_DT = mybir.dt.bfloat16
OUT_DT = mybir.dt.bfloat16

_CACHE = {}


def _fold_weights(e1_w, e2_w, w_vals, b_vals, e1_parent, e2_parent,
                  w_rows, w_cols, b_idx):
    """Host-side weight preprocessing: densify W/b and fold the two embed
    layers' contributions into (cols, signs, A) so the device computes
    out = x @ W0t + relu(sign*x[:, cols]) @ A + bias."""
    W = np.bincount(w_rows.astype(np.int64) * F2 + w_cols.astype(np.int64),
                    weights=w_vals.astype(np.float64),
                    minlength=O * F2).reshape(O, F2)
    bias = np.bincount(b_idx.astype(np.int64), weights=b_vals.astype(np.float64),
                       minlength=O)
    W0t = W[:, :F0].T          # [2048, 1024]
    W1t = W[:, F0:F1].T        # [256, 1024]  layer-1 embed rows
    W2t = W[:, F1:F2].T        # [256, 1024]  layer-2 embed rows

    # each embed column j contributes scale*relu(s*x[:, c]) with weight row w
    # accumulate per (c, s): A_map[(c, s)] += scale * w_row
    A_map = {}

    def acc(c, s, scale, wrow):
        if scale == 0.0:
            return
        key = (int(c), int(s))
        if key in A_map:
            A_map[key] = A_map[key] + scale * wrow
        else:
            A_map[key] = scale * wrow

    e1_parent = e1_parent.astype(np.int64)
    e2_parent = e2_parent.astype(np.int64)
    e1_w64 = e1_w.astype(np.float64)
    e2_w64 = e2_w.astype(np.float64)

    for j in range(E1):
        w = e1_w64[j]
        s = 1 if w >= 0 else -1
        acc(e1_parent[j], s, abs(w), W1t[j])
    for j in range(E2):
        q = e2_parent[j]
        w = e2_w64[j]
        if q < F0:
            s = 1 if w >= 0 else -1
            acc(q, s, abs(w), W2t[j])
        else:
            # refers to layer-1 embed column m1: h1e[:, m1] >= 0 always
            if w < 0:
                continue  # relu(negative * nonneg) == 0
            m1 = q - F0
            w1 = e1_w64[m1]
            s = 1 if w1 >= 0 else -1
            acc(e1_parent[m1], s, w * abs(w1), W2t[j])

    pairs = sorted(A_map.keys())
    n_pairs = len(pairs)
    # relu-block k-tiles; last row of the block is reserved for the bias row
    RT = max(1, -(-(n_pairs + 1) // P))
    n_rows = RT * P
    cols = np.zeros(n_rows, dtype=np.int64)
    signs = np.ones(n_rows, dtype=np.float32)
    A = np.zeros((n_rows, O), dtype=np.float64)
    for i, (c, s) in enumerate(pairs):
        cols[i] = c
        signs[i] = s
        A[i] = A_map[(c, s)]
    return (W0t.astype(np.float32), A.astype(np.float32),
            bias.astype(np.float32), cols, signs, RT)


def _build_program(RT):
    """Build + compile the SPMD Bass program (same for every core)."""
    KT = KT_X + RT  # total k-tiles
    MDT = MATMUL_DT
    nc = bacc.Bacc("TRN2", target_bir_lowering=False, debug=False,
                   num_devices=N_CORES)

    xt_d = nc.dram_tensor("xt", [KT_X, P, BS], MDT, kind="ExternalInput")
    xg_d = nc.dram_tensor("xg", [RT, P, BS], MDT, kind="ExternalInput")
    wc_d = nc.dram_tensor("wc", [KT, P, O], MDT, kind="ExternalInput")
    sg_d = nc.dram_tensor("sg", [P, RT], mybir.dt.float32,
                          kind="ExternalInput")
    # [m, p, c]: batch row = m*128 + p, so a flat reshape on the host works
    out_d = nc.dram_tensor("out", [MT, P, O], OUT_DT, kind="ExternalOutput")

    with tile.TileContext(nc) as tc:
        with (
            tc.tile_pool(name="sbuf", bufs=1) as pool,
            tc.tile_pool(name="outp", bufs=1) as outp,
            tc.tile_pool(name="psum", bufs=8, space="PSUM") as psum,
        ):
            sg_sb = pool.tile([P, RT], mybir.dt.float32, tag="sg")
            nc.sync.dma_start(sg_sb[:], sg_d[:])

            # PE warm-up from a memset tile (no DMA dependency): keeps the
            # PE busy from engine-ready so the HAM clock gate flips to
            # 2.4 GHz as early as possible (cold matmuls run at 1.2 GHz)
            wsrc = pool.tile([P, 256], MDT, tag="wrm", name="wrm")
            nc.vector.memset(wsrc[:], 0.25)
            wps = psum.tile([P, N_HALF], mybir.dt.float32, tag="ps",
                            name="wps")
            for _ in range(WARMUP_MMS):
                nc.tensor.matmul(wps[:, :256], wsrc[:, :P], wsrc[:],
                                 start=True, stop=True)

            # input streams: lhsT k-tiles on the sync queue, weight k-tiles
            # on the scalar queue (parallel HWDGE queues halve the time to
            # first tile and double descriptor throughput). kt0 goes in
            # small chunks so the PE's first real matmul starts ASAP.
            lh = [pool.tile([P, BS], MDT, tag=f"x{kt}", name=f"x{kt}")
                  for kt in range(KT_X)]
            wc = [pool.tile([P, O], MDT, tag=f"w{kt}", name=f"w{kt}")
                  for kt in range(KT)]
            nc.sync.dma_start(lh[0][:, :256], xt_d[0][:, :256])
            nc.scalar.dma_start(wc[0][:, :N_HALF], wc_d[0][:, :N_HALF])
            nc.sync.dma_start(lh[0][:, 256:512], xt_d[0][:, 256:512])
            nc.scalar.dma_start(wc[0][:, N_HALF:], wc_d[0][:, N_HALF:])
            nc.sync.dma_start(lh[0][:, 512:], xt_d[0][:, 512:])
            for kt in range(1, KT):
                if kt < KT_X:
                    nc.sync.dma_start(lh[kt][:], xt_d[kt])
                nc.scalar.dma_start(wc[kt][:], wc_d[kt])
            # gathered relu-source columns after the main lhsT stream (they
            # are consumed last); bufs=RT so no ring wait can head-of-line
            # block the in-order sync queue
            for t in range(RT):
                g_sb = pool.tile([P, BS], MDT, tag="g",
                                 name=f"g{t}", bufs=RT)
                nc.sync.dma_start(g_sb[:], xg_d[t])
                r_sb = pool.tile([P, BS], MDT, tag=f"r{t}", name=f"r{t}")
                # bias row: xg's last row is all-ones with sign +1, so the
                # sign-relu passes it through unchanged
                nc.vector.tensor_scalar(r_sb[:], g_sb[:],
                                        sg_sb[:, t:t + 1], 0.0,
                                        mybir.AluOpType.mult,
                                        mybir.AluOpType.max)
                lh.append(r_sb)

            # Two passes of 8 groups, each group = one PSUM bank holding the
            # full-K accumulation for a [128m x 512n] output block.
            o_sbs = [outp.tile([P, O], OUT_DT, tag=f"o{m}", name=f"o{m}")
                     for m in range(MT)]

            def drain(m, n, ps):
                osl = o_sbs[m][:, n * N_HALF:(n + 1) * N_HALF]
                nc.vector.tensor_copy(osl, ps[:])
                eng = nc.sync if n == 0 else nc.scalar
                eng.dma_start(out_d[m][:, n * N_HALF:(n + 1) * N_HALF], osl)

            def run_pass(groups, k_major_upto):
                pss = {g: psum.tile([P, N_HALF], mybir.dt.float32,
                                    tag="ps", name="ps") for g in groups}
                # k-major phase: consume k-tiles in DMA arrival order
                for kt in range(k_major_upto):
                    for (m, n) in groups:
                        nc.tensor.matmul(
                            pss[(m, n)][:],
                            lh[kt][:, m * P:(m + 1) * P],
                            wc[kt][:, n * N_HALF:(n + 1) * N_HALF],
                            start=(kt == 0), stop=False)
                # group-major endgame: groups finish staggered so drains +
                # stores pipeline behind the PE instead of piling up
                for (m, n) in groups:
                    for kt in range(k_major_upto, KT):
                        nc.tensor.matmul(
                            pss[(m, n)][:],
                            lh[kt][:, m * P:(m + 1) * P],
                            wc[kt][:, n * N_HALF:(n + 1) * N_HALF],
                            start=(kt == 0), stop=(kt == KT - 1))
                    drain(m, n, pss[(m, n)])

            G1 = [(m, n) for m in range(4) for n in range(2)]
            G2 = [(m, n) for m in range(4, 8) for n in range(2)]
            run_pass(G1, KT_X)   # k-major while the stream is in flight
            run_pass(G2, 0)      # fully resident: pure group-major

    nc.compile()
    return nc


def kernel(input, e1_w, e2_w, w_vals, b_vals, e1_parent, e2_parent,
           w_rows, w_cols, b_idx):
    input = np.asarray(input, dtype=np.float32)
    W0t, A, bias, cols, signs, RT = _fold_weights(
        np.asarray(e1_w), np.asarray(e2_w), np.asarray(w_vals),
        np.asarray(b_vals), np.asarray(e1_parent), np.asarray(e2_parent),
        np.asarray(w_rows), np.asarray(w_cols), np.asarray(b_idx))

    KT = KT_X + RT
    # weight slab: [KT*128, O] = [W0t ; A-with-bias-row]
    wc = np.concatenate([W0t, A], axis=0)
    wc[KT * P - 1, :] = bias           # lhsT row is all-ones -> adds bias
    wc = np.ascontiguousarray(wc.reshape(KT, P, O), dtype=np.float32)
    sg = np.ascontiguousarray(signs.reshape(RT, P).T, dtype=np.float32)

    key = (RT, MATMUL_DT)
    if key not in _CACHE:
        _CACHE[key] = _build_program(RT)
    nc = _CACHE[key]

    xg_full = input[:, cols]           # [B, RT*128] gathered source columns
    xg_full[:, RT * P - 1] = 1.0       # all-ones bias column (sign is +1)
    import ml_dtypes
    bf = np.dtype(ml_dtypes.bfloat16)
    xmm = input.astype(bf)
    xg_full = xg_full.astype(bf)
    wc = wc.astype(bf)
    in_maps = []
    for c in range(N_CORES):
        sl = slice(c * BS, (c + 1) * BS)
        xt_c = np.ascontiguousarray(xmm[sl].T.reshape(KT_X, P, BS))
        xg_c = np.ascontiguousarray(xg_full[sl].T.reshape(RT, P, BS))
        in_maps.append({"xt": xt_c, "xg": xg_c, "wc": wc, "sg": sg})

    res = run_bass_kernel_spmd(nc, in_maps, list(range(N_CORES)))
    out = np.concatenate(
        [np.asarray(res.results[c]["out"]).astype(np.float32).reshape(BS, O)
         for c in range(N_CORES)], axis=0)
    return out
